# revision 1
# baseline (speedup 1.0000x reference)
"""Trainium2 Bass kernel for nn_CNNStateEncoder (dense_cnn).

Network per row (B*S rows, 8 features each):
  conv1 2x2 on [1,2,4] -> 32ch x [1,3]   == h1[96]  = A1[96,8]  @ x[8],  relu(+b1)
  conv2 1x2 on [32,1,3] -> 32ch x [1,2]  == h2[64]  = A2[64,96] @ h1,    relu(+b2)
  linear 64->64                          == out[64] = Wp[64,64] @ h2 + bp

Mapping on each NeuronCore (data parallel over 8 cores, 65536 rows/core,
2048-row tiles; PE HAM stays at 1.2GHz on this part, so minimize matmul
count and maximize row/col-group concurrency):
  - rows live in the matmul free dim (feature-major chain)
  - input: DVE cast f32->bf16, gpsimd x4-replicate into 32-blocks, DVE
    StreamTranspose; row-chunk q's 8 features land at partitions 32q..32q+8
  - conv1: 4 concurrently-packed K=8 matmuls (row groups), one psum bank
    each (concurrent drains must hit distinct banks)
  - relu1: ONE contiguous ACT op over the 4 banks
  - conv2: K=96, N=512 matmuls; the tile's two 1024-row halves go to output
    col groups 0/64 and run concurrently (packed by partition halves)
  - linear: lhsT = activations (M=rows) -> row-major PSUM; 16 chunks issued
    as concurrent (row-group 0-1 x bank0, row-group 2-3 x bank1) pairs
  - out: single DVE bias-add+copy, single 512KB store
"""

import numpy as np
import ml_dtypes

B, S, FEAT, OUT = 64, 8192, 8, 64
NCORES = 8
ROWS_TOTAL = B * S
ROWS_CORE = ROWS_TOTAL // NCORES  # 65536
TILE_ROWS = 2048

BF16 = ml_dtypes.bfloat16

# ---------------------------------------------------------------------------
# numpy-side weight packing
# ---------------------------------------------------------------------------

def pack_weights(W1, b1, W2, b2, Wp, bp):
    W1 = np.asarray(W1, np.float32)
    W2 = np.asarray(W2, np.float32)
    Wp = np.asarray(Wp, np.float32)
    b1 = np.asarray(b1, np.float32)
    b2 = np.asarray(b2, np.float32)
    bp = np.asarray(bp, np.float32)

    # A1 [96, 8]: h1[o*3+j] = sum_{kh,kw} x[kh*4 + j + kw] * W1[o,0,kh,kw]
    A1 = np.zeros((96, 8), np.float32)
    for o in range(32):
        for j in range(3):
            for kh in range(2):
                for kw in range(2):
                    A1[o * 3 + j, kh * 4 + j + kw] += W1[o, 0, kh, kw]
    b1_96 = np.repeat(b1, 3).astype(np.float32)

    # A2 [64, 96]: h2[c*2+w] = sum_{i,kw} h1[i*3 + w + kw] * W2[c,i,0,kw]
    A2 = np.zeros((64, 96), np.float32)
    for c in range(32):
        for w in range(2):
            for i in range(32):
                for kw in range(2):
                    A2[c * 2 + w, i * 3 + w + kw] += W2[c, i, 0, kw]
    b2_64 = np.repeat(b2, 2).astype(np.float32)

    a1t = np.zeros((128, 96), np.float32)
    for q in range(4):
        a1t[32 * q:32 * q + 8, :] = A1.T
    a2t = np.zeros((96, 128), np.float32)
    a2t[:, 0:64] = A2.T
    a2t[:, 64:128] = A2.T
    wpt = np.zeros((128, 64), np.float32)
    wpt[0:64, :] = Wp.T
    wpt[64:128, :] = Wp.T
    b1c = b1_96.reshape(96, 1)
    b2c = np.concatenate([b2_64, b2_64]).reshape(128, 1)
    bpb = np.tile(bp, (128, TILE_ROWS // 128))  # [128, 1024]

    return {
        "a1t": a1t.astype(BF16),
        "a2t": a2t.astype(BF16),
        "wpt": wpt.astype(BF16),
        "b1c": b1c,
        "b2c": b2c,
        "bpb": bpb.astype(np.float32),
    }


# ---------------------------------------------------------------------------
# bass module
# ---------------------------------------------------------------------------

def build_nc(rows=ROWS_CORE):
    import concourse.bass as bass
    import concourse.bacc as bacc
    import concourse.mybir as mybir
    import concourse.tile as tile

    f32 = mybir.dt.float32
    bf16 = mybir.dt.bfloat16
    Relu = mybir.ActivationFunctionType.Relu
    Alu = mybir.AluOpType

    assert rows % TILE_ROWS == 0
    ntiles = rows // TILE_ROWS

    nc = bacc.Bacc(None, target_bir_lowering=False)

    x_d = nc.dram_tensor("x", [rows, FEAT], f32, kind="ExternalInput")
    a1t_d = nc.dram_tensor("a1t", [128, 96], bf16, kind="ExternalInput")
    a2t_d = nc.dram_tensor("a2t", [96, 128], bf16, kind="ExternalInput")
    wpt_d = nc.dram_tensor("wpt", [128, 64], bf16, kind="ExternalInput")
    b1c_d = nc.dram_tensor("b1c", [96, 1], f32, kind="ExternalInput")
    b2c_d = nc.dram_tensor("b2c", [128, 1], f32, kind="ExternalInput")
    bpb_d = nc.dram_tensor("bpb", [128, 1024], f32, kind="ExternalInput")
    out_d = nc.dram_tensor("out", [rows, OUT], f32, kind="ExternalOutput")

    with tile.TileContext(nc) as tc:
        with (
            tc.tile_pool(name="consts", bufs=1) as cpool,
            tc.tile_pool(name="xin", bufs=4) as xpool,
            tc.tile_pool(name="xbf", bufs=4) as xbpool,
            tc.tile_pool(name="xpad", bufs=4) as xppool,
            tc.tile_pool(name="xt", bufs=4) as xtpool,
            tc.tile_pool(name="h1s", bufs=3) as h1pool,
            tc.tile_pool(name="h2s", bufs=3) as h2pool,
            tc.tile_pool(name="osb", bufs=3) as opool,
            tc.tile_pool(name="ps_h1", bufs=1, space="PSUM") as ps_h1,
            tc.tile_pool(name="ps_h2", bufs=1, space="PSUM") as ps_h2,
            tc.tile_pool(name="ps_o", bufs=1, space="PSUM") as ps_o,
        ):
            a1t = cpool.tile([128, 96], bf16)
            a2t = cpool.tile([96, 128], bf16)
            wpt = cpool.tile([128, 64], bf16)
            b1c = cpool.tile([96, 1], f32)
            b2c = cpool.tile([128, 1], f32)
            bpb = cpool.tile([128, 1024], f32)
            nc.sync.dma_start(a1t[:], a1t_d[:])
            nc.sync.dma_start(a2t[:], a2t_d[:])
            nc.sync.dma_start(wpt[:], wpt_d[:])
            nc.sync.dma_start(b1c[:], b1c_d[:])
            nc.sync.dma_start(b2c[:], b2c_d[:])
            nc.sync.dma_start(bpb[:], bpb_d[:])

            for t in range(ntiles):
                n0 = t * TILE_ROWS
                # ---- load + cast + replicate + transpose ----
                x_sb = xpool.tile([128, 128], f32)
                nc.sync.dma_start(
                    x_sb[:],
                    x_d[n0:n0 + TILE_ROWS, :].rearrange("(p r) f -> p (r f)", p=128),
                )
                x_bf = xbpool.tile([128, 128], bf16)
                nc.vector.tensor_copy(x_bf[:], x_sb[:])
                # x_pad[p, 32a+8g+f] = x_bf[p, 8a+f] = x[n0 + 16p + a, f]
                x_pad = xppool.tile([128, 512], bf16)
                rep_ap = (
                    x_bf[:]
                    .rearrange("p (a f) -> p a f", f=8)
                    .unsqueeze(2)
                    .broadcast_to((128, 16, 4, 8))
                )
                nc.gpsimd.tensor_copy(x_pad[:], rep_ap)
                # xt[32q+8g+f, 32a+v] = x[n0 + 512q + 16v + a, f]
                xt = xtpool.tile([128, 512], bf16)
                nc.vector.transpose(xt[:], x_pad[:])

                # ---- conv1: 4 packed K=8 matmuls, one psum bank each ----
                # rhs streams (v outer, a inner) so bank q's col j = row
                # n0 + 512q + j
                h1ps = ps_h1.tile([96, 2048], f32)
                for q in range(4):
                    rhs = xt[32 * q:32 * q + 8, :].rearrange("k (a v) -> k v a", v=32)
                    nc.tensor.matmul(
                        h1ps[:, 512 * q:512 * q + 512],
                        a1t[32 * q:32 * q + 8, :],
                        rhs,
                        tile_position=(32 * q, 0),
                    )
                # ---- relu1 (+b1): ONE contiguous ACT op ----
                h1s = h1pool.tile([96, 2048], bf16)
                nc.scalar.activation(h1s[:], h1ps[:], Relu, bias=b1c[:])

                # ---- conv2: 4 matmuls; the two 1024-row halves of the tile
                # land on col groups 0/64 and run concurrently ----
                h2ps_a = ps_h2.tile([128, 512], f32)
                h2ps_b = ps_h2.tile([128, 512], f32)
                for ps, lo in ((h2ps_a, 0), (h2ps_b, 512)):
                    for h in (0, 1):
                        nc.tensor.matmul(
                            ps[64 * h:64 * h + 64, :],
                            a2t[:, 64 * h:64 * h + 64],
                            h1s[:, 1024 * h + lo:1024 * h + lo + 512],
                            tile_position=(0, 64 * h),
                        )
                # ---- relu2 (+b2): bank A on ACT, bank B on DVE ----
                h2s_a = h2pool.tile([128, 512], bf16)
                h2s_b = h2pool.tile([128, 512], bf16)
                nc.scalar.activation(h2s_a[:], h2ps_a[:], Relu, bias=b2c[:])
                nc.vector.tensor_scalar(
                    h2s_b[:], h2ps_b[:], b2c[:], 0.0, Alu.add, Alu.max
                )

                # ---- linear: 16 chunks of 128 rows; issue (h=0, h=1) chunk
                # pairs adjacently -> concurrent row groups + distinct banks.
                # chunk c covers rows [n0+128c, +128); h = c//8 selects the
                # h2 partition half, bank = c//8 too (cols 64c).
                outps = ps_o.tile([128, 1024], f32)
                for cc in range(8):
                    for h in (0, 1):
                        c = 8 * h + cc
                        X = (c // 4) % 2
                        h2s = h2s_a if X == 0 else h2s_b
                        col = 128 * (c % 4)
                        nc.tensor.matmul(
                            outps[:, 64 * c:64 * c + 64],
                            h2s[64 * h:64 * h + 64, col:col + 128],
                            wpt[64 * h:64 * h + 64, :],
                            start=(cc == 0),
                            stop=(cc == 7),
                            tile_position=(64 * h, 0),
                        )
                # ---- bias + store ----
                out_sb = opool.tile([128, 1024], f32)
                nc.vector.tensor_tensor(out_sb[:], outps[:], bpb[:], Alu.add)
                nc.sync.dma_start(
                    out_d[n0:n0 + TILE_ROWS, :].rearrange("(c p) j -> p c j", p=128),
                    out_sb[:],
                )

    nc.compile()
    return nc


# ---------------------------------------------------------------------------
# entry point
# ---------------------------------------------------------------------------

_CACHE = {}


def _get_nc(rows=ROWS_CORE):
    if rows not in _CACHE:
        _CACHE[rows] = build_nc(rows)
    return _CACHE[rows]


def kernel(x, W1, b1, W2, b2, Wp, bp):
    from concourse.bass_utils import run_bass_kernel_spmd

    x = np.ascontiguousarray(np.asarray(x, np.float32)).reshape(ROWS_TOTAL, FEAT)
    consts = pack_weights(W1, b1, W2, b2, Wp, bp)

    nc = _get_nc()
    in_maps = []
    for c in range(NCORES):
        m = dict(consts)
        m["x"] = x[c * ROWS_CORE:(c + 1) * ROWS_CORE]
        in_maps.append(m)

    res = run_bass_kernel_spmd(nc, in_maps, core_ids=list(range(NCORES)))
    out = np.concatenate([r["out"] for r in res.results], axis=0)
    return out.reshape(B, S, OUT)



# revision 3
# speedup vs baseline: 1.3257x; 1.3257x over previous
"""Trainium2 Bass kernel for nn_CNNStateEncoder (dense_cnn).

Network per row (B*S rows, 8 features each):
  conv1 2x2 on [1,2,4] -> 32ch x [1,3]   == h1[96]  = A1[96,8]  @ x[8],  relu(+b1)
  conv2 1x2 on [32,1,3] -> 32ch x [1,2]  == h2[64]  = A2[64,96] @ h1,    relu(+b2)
  linear 64->64                          == out[64] = Wp[64,64] @ h2 + bp

Per-core mapping (data parallel, 65536 rows/core, feature-major all the way):
  - host pre-transposes x to xT8 [8, 65536] fp16 -> no on-device cast/
    replicate/transpose at all; loads are plain strided HWDGE DMAs
  - conv1: 4 row-strip-packed K=8 matmuls (tile_position (32q,0)), N=512,
    one fp32 psum bank each -> h1ps [96, 2048] per 2048-row tile
  - relu1+b1: single DVE tensor_scalar (psum->sbuf fp16)
  - conv2: K=96 col-packed pairs (0,0)/(0,64), N=512 -> h2ps [128, 1024]
    (parts 0-63 / 64-127 hold different 512-row blocks, shared bank is ok
    for disjoint partition ranges)
  - relu2+b2: single ACT activation (bias operand is free)
  - linear: Wp stationary, h2 moving -> out stays feature-major
    ((0,0)+(64,64) col/row-packed pairs); out bias via ACT Identity+bias
  - out evac fp16, 1MB batched stores of out_d [128, 32768];
    host undoes the (block, tile, parity) column permutation + casts f32
  - everything fp16 on the wire (tolerance 2e-2; measured err ~7e-4)
"""

import numpy as np

B, S, FEAT, OUT = 64, 8192, 8, 64
NCORES = 8
ROWS_TOTAL = B * S
ROWS_CORE = ROWS_TOTAL // NCORES  # 65536
TILE = 2048                       # rows per psum tile
NT = ROWS_CORE // TILE            # 32 tiles
TPB = 4                           # tiles per dma batch
NB = NT // TPB                    # 8 batches
QCH = ROWS_CORE // 4              # 16384 rows per conv1 strip-block

F16 = np.float16

# ---------------------------------------------------------------------------
# numpy-side packing
# ---------------------------------------------------------------------------

def pack_weights(W1, b1, W2, b2, Wp, bp):
    W1 = np.asarray(W1, np.float32)
    W2 = np.asarray(W2, np.float32)
    Wp = np.asarray(Wp, np.float32)
    b1 = np.asarray(b1, np.float32)
    b2 = np.asarray(b2, np.float32)
    bp = np.asarray(bp, np.float32)

    # A1 [96, 8]: h1[o*3+j] = sum_{kh,kw} x[kh*4 + j + kw] * W1[o,0,kh,kw]
    A1 = np.zeros((96, 8), np.float32)
    for o in range(32):
        for j in range(3):
            for kh in range(2):
                for kw in range(2):
                    A1[o * 3 + j, kh * 4 + j + kw] += W1[o, 0, kh, kw]
    b1_96 = np.repeat(b1, 3).astype(np.float32)

    # A2 [64, 96]: h2[c*2+w] = sum_{i,kw} h1[i*3 + w + kw] * W2[c,i,0,kw]
    A2 = np.zeros((64, 96), np.float32)
    for c in range(32):
        for w in range(2):
            for i in range(32):
                for kw in range(2):
                    A2[c * 2 + w, i * 3 + w + kw] += W2[c, i, 0, kw]
    b2_64 = np.repeat(b2, 2).astype(np.float32)

    a1t = np.zeros((128, 96), F16)
    for q in range(4):
        a1t[32 * q:32 * q + 8, :] = A1.T.astype(F16)
    a2t = A2.T.astype(F16)                      # [96, 64]
    wpt = np.zeros((128, 64), F16)
    wpt[0:64, :] = Wp.T.astype(F16)
    wpt[64:128, :] = Wp.T.astype(F16)
    b1c = b1_96.reshape(96, 1)
    b2c = np.concatenate([b2_64, b2_64]).reshape(128, 1)
    bpc = np.concatenate([bp, bp]).reshape(128, 1)
    return {"a1t": a1t, "a2t": a2t, "wpt": wpt,
            "b1c": b1c, "b2c": b2c, "bpc": bpc}


def build_in_maps(x, W1, b1, W2, b2, Wp, bp):
    x = np.ascontiguousarray(np.asarray(x, np.float32)).reshape(ROWS_TOTAL, FEAT)
    consts = pack_weights(W1, b1, W2, b2, Wp, bp)
    in_maps = []
    for c in range(NCORES):
        xc = x[c * ROWS_CORE:(c + 1) * ROWS_CORE]
        m = dict(consts)
        m["xT8"] = np.ascontiguousarray(xc.T.astype(F16))  # [8, 65536]
        in_maps.append(m)
    return in_maps


def reconstruct(results):
    outs = []
    for r in results:
        od = np.asarray(r["out"])  # [128, 32768] fp16
        a = od.astype(np.float32).reshape(2, 64, NB, TPB, 2, 512)  # (h,f,T,s,jH,i)
        a = a.transpose(4, 0, 2, 3, 5, 1)                          # (jH,h,T,s,i,f)
        outs.append(a.reshape(ROWS_CORE, OUT))
    return np.concatenate(outs, 0).reshape(B, S, OUT)


# ---------------------------------------------------------------------------
# bass module
# ---------------------------------------------------------------------------

def build_nc():
    import concourse.bass as bass
    import concourse.bacc as bacc
    import concourse.mybir as mybir
    import concourse.tile as tile

    f32 = mybir.dt.float32
    f16 = mybir.dt.float16
    Relu = mybir.ActivationFunctionType.Relu
    Ident = mybir.ActivationFunctionType.Identity
    Alu = mybir.AluOpType

    nc = bacc.Bacc(None, target_bir_lowering=False)

    xT8_d = nc.dram_tensor("xT8", [FEAT, ROWS_CORE], f16, kind="ExternalInput")
    a1t_d = nc.dram_tensor("a1t", [128, 96], f16, kind="ExternalInput")
    a2t_d = nc.dram_tensor("a2t", [96, 64], f16, kind="ExternalInput")
    wpt_d = nc.dram_tensor("wpt", [128, 64], f16, kind="ExternalInput")
    b1c_d = nc.dram_tensor("b1c", [96, 1], f32, kind="ExternalInput")
    b2c_d = nc.dram_tensor("b2c", [128, 1], f32, kind="ExternalInput")
    bpc_d = nc.dram_tensor("bpc", [128, 1], f32, kind="ExternalInput")
    out_d = nc.dram_tensor("out", [128, ROWS_CORE // 2], f16, kind="ExternalOutput")

    with tile.TileContext(nc) as tc:
        with (
            tc.tile_pool(name="consts", bufs=1) as cpool,
            tc.tile_pool(name="xin", bufs=2) as xpool,
            tc.tile_pool(name="h1s", bufs=2) as h1pool,
            tc.tile_pool(name="h2s", bufs=2) as h2pool,
            tc.tile_pool(name="osb", bufs=2) as opool,
            tc.tile_pool(name="ps_h1", bufs=1, space="PSUM") as ps_h1,
            tc.tile_pool(name="ps_h2", bufs=1, space="PSUM") as ps_h2,
            tc.tile_pool(name="ps_o", bufs=1, space="PSUM") as ps_o,
        ):
            a1t = cpool.tile([128, 96], f16)
            a2t = cpool.tile([96, 64], f16)
            wpt = cpool.tile([128, 64], f16)
            b1c = cpool.tile([96, 1], f32)
            b2c = cpool.tile([128, 1], f32)
            bpc = cpool.tile([128, 1], f32)
            nc.sync.dma_start(a1t[:], a1t_d[:])
            nc.sync.dma_start(a2t[:], a2t_d[:])
            nc.sync.dma_start(wpt[:], wpt_d[:])
            nc.sync.dma_start(b1c[:], b1c_d[:])
            nc.sync.dma_start(b2c[:], b2c_d[:])
            nc.sync.dma_start(bpc[:], bpc_d[:])

            for T in range(NB):
                # ---- input batch: parts 32q..32q+8 <- block q rows ----
                xt = xpool.tile([128, TPB * 512], f16)
                for q in range(4):
                    nc.sync.dma_start(
                        xt[32 * q:32 * q + 8, :],
                        xT8_d[:, QCH * q + 2048 * T:QCH * q + 2048 * T + 2048],
                    )
                outsb = opool.tile([128, TPB * 1024], f16)
                for s in range(TPB):
                    # ---- conv1: 4 row-strip-packed K=8 matmuls ----
                    h1ps = ps_h1.tile([96, 2048], f32)
                    for q in range(4):
                        nc.tensor.matmul(
                            h1ps[:, 512 * q:512 * q + 512],
                            a1t[32 * q:32 * q + 8, :],
                            xt[32 * q:32 * q + 8, 512 * s:512 * s + 512],
                            tile_position=(32 * q, 0),
                        )
                    # ---- relu1 + b1 on DVE ----
                    h1s = h1pool.tile([96, 2048], f16)
                    nc.vector.tensor_scalar(
                        h1s[:], h1ps[:], b1c[:], 0.0, Alu.add, Alu.max
                    )
                    # ---- conv2: col-packed (0,0)/(0,64) pairs, K=96 ----
                    h2ps = ps_h2.tile([128, 1024], f32)
                    for r in range(2):
                        nc.tensor.matmul(
                            h2ps[0:64, 512 * r:512 * r + 512],
                            a2t[:],
                            h1s[:, 1024 * r:1024 * r + 512],
                            tile_position=(0, 0),
                        )
                        nc.tensor.matmul(
                            h2ps[64:128, 512 * r:512 * r + 512],
                            a2t[:],
                            h1s[:, 1024 * r + 512:1024 * r + 1024],
                            tile_position=(0, 64),
                        )
                    # ---- relu2 + b2 on ACT ----
                    h2s = h2pool.tile([128, 1024], f16)
                    nc.scalar.activation(h2s[:], h2ps[:], Relu, bias=b2c[:])
                    # ---- linear: (0,0)+(64,64) packed pairs, K=64 ----
                    outps = ps_o.tile([128, 1024], f32)
                    for r in range(2):
                        nc.tensor.matmul(
                            outps[0:64, 512 * r:512 * r + 512],
                            wpt[0:64, :],
                            h2s[0:64, 512 * r:512 * r + 512],
                            tile_position=(0, 0),
                        )
                        nc.tensor.matmul(
                            outps[64:128, 512 * r:512 * r + 512],
                            wpt[64:128, :],
                            h2s[64:128, 512 * r:512 * r + 512],
                            tile_position=(64, 64),
                        )
                    # ---- out + bp on ACT ----
                    nc.scalar.activation(
                        outsb[:, 1024 * s:1024 * s + 1024], outps[:], Ident,
                        bias=bpc[:],
                    )
                # ---- batched 1MB store ----
                nc.sync.dma_start(
                    out_d[:, 4096 * T:4096 * T + 4096], outsb[:]
                )

    nc.compile()
    return nc


# ---------------------------------------------------------------------------
# entry point
# ---------------------------------------------------------------------------

_CACHE = {}


def _get_nc():
    if "nc" not in _CACHE:
        _CACHE["nc"] = build_nc()
    return _CACHE["nc"]


def kernel(x, W1, b1, W2, b2, Wp, bp):
    from concourse.bass_utils import run_bass_kernel_spmd

    nc = _get_nc()
    in_maps = build_in_maps(x, W1, b1, W2, b2, Wp, bp)
    res = run_bass_kernel_spmd(nc, in_maps, core_ids=list(range(NCORES)))
    return reconstruct(res.results)


# revision 5
# speedup vs baseline: 1.4221x; 1.0727x over previous
"""Trainium2 Bass kernel for nn_CNNStateEncoder (dense_cnn).

Network per row (B*S rows, 8 features each):
  conv1 2x2 on [1,2,4] -> 32ch x [1,3]   == h1[96]  = A1[96,8]  @ x[8],  relu(+b1)
  conv2 1x2 on [32,1,3] -> 32ch x [1,2]  == h2[64]  = A2[64,96] @ h1,    relu(+b2)
  linear 64->64                          == out[64] = Wp[64,64] @ h2 + bp

Per-core mapping (data parallel, 65536 rows/core, feature-major all the way):
  - host pre-transposes x to xT8 [8, 65536] fp16; rows split into two
    32768-row half-streams (PE row strips 0 and 1) -> plain strided DMAs
  - 1024-row tiles, 4 PSUM banks each (h1 2 + h2 1 + out 1), double
    buffered -> software-pipelined loop: PE issues conv1(t), conv2(t-1),
    linear(t-2) each iteration so it never head-of-line blocks and HAM
    stays warm (2.4GHz)
  - conv1: 2 strip-packed K=8 matmuls N=512 (tile_position (0,0)/(32,0))
  - relu1+b1: DVE tensor_scalar  [96,1024]  (psum->sbuf fp16)
  - conv2: K=96 col-packed pair (0,0)/(0,64) N=512 -> h2ps [128,512]
    (parts 0-63 = half-stream 0 rows, 64-127 = half-stream 1)
  - relu2+b2: ACT activation [128,512]
  - linear: Wp stationary, h2 moving, (0,0)+(64,64) pair -> feature-major
  - out+bp: ACT Identity+bias [128,512]; 1MB batched stores
  - host undoes the (half-stream, tile) permutation + casts f32
"""

import numpy as np

B, S, FEAT, OUT = 64, 8192, 8, 64
NCORES = 8
ROWS_TOTAL = B * S
ROWS_CORE = ROWS_TOTAL // NCORES   # 65536
TILE = 1024                        # rows per psum tile
NT = ROWS_CORE // TILE             # 64 tiles
TPB = 8                            # tiles per dma batch
NB = NT // TPB                     # 8 batches
HALF = ROWS_CORE // 2              # 32768 rows per strip-block

F16 = np.float16

# ---------------------------------------------------------------------------
# numpy-side packing
# ---------------------------------------------------------------------------

def pack_weights(W1, b1, W2, b2, Wp, bp):
    W1 = np.asarray(W1, np.float32)
    W2 = np.asarray(W2, np.float32)
    Wp = np.asarray(Wp, np.float32)
    b1 = np.asarray(b1, np.float32)
    b2 = np.asarray(b2, np.float32)
    bp = np.asarray(bp, np.float32)

    # A1 [96, 8]: h1[o*3+j] = sum_{kh,kw} x[kh*4 + j + kw] * W1[o,0,kh,kw]
    A1 = np.zeros((96, 8), np.float32)
    for o in range(32):
        for j in range(3):
            for kh in range(2):
                for kw in range(2):
                    A1[o * 3 + j, kh * 4 + j + kw] += W1[o, 0, kh, kw]
    b1_96 = np.repeat(b1, 3).astype(np.float32)

    # A2 [64, 96]: h2[c*2+w] = sum_{i,kw} h1[i*3 + w + kw] * W2[c,i,0,kw]
    A2 = np.zeros((64, 96), np.float32)
    for c in range(32):
        for w in range(2):
            for i in range(32):
                for kw in range(2):
                    A2[c * 2 + w, i * 3 + w + kw] += W2[c, i, 0, kw]
    b2_64 = np.repeat(b2, 2).astype(np.float32)

    a1t = np.zeros((64, 96), F16)
    for q in range(2):
        a1t[32 * q:32 * q + 8, :] = A1.T.astype(F16)
    a2t = A2.T.astype(F16)                      # [96, 64]
    wpt = np.zeros((128, 64), F16)
    wpt[0:64, :] = Wp.T.astype(F16)
    wpt[64:128, :] = Wp.T.astype(F16)
    b1c = b1_96.reshape(96, 1)
    b2c = np.concatenate([b2_64, b2_64]).reshape(128, 1)
    bpc = np.concatenate([bp, bp]).reshape(128, 1)
    return {"a1t": a1t, "a2t": a2t, "wpt": wpt,
            "b1c": b1c, "b2c": b2c, "bpc": bpc}


def build_in_maps(x, W1, b1, W2, b2, Wp, bp):
    x = np.ascontiguousarray(np.asarray(x, np.float32)).reshape(ROWS_TOTAL, FEAT)
    consts = pack_weights(W1, b1, W2, b2, Wp, bp)
    in_maps = []
    for c in range(NCORES):
        xc = x[c * ROWS_CORE:(c + 1) * ROWS_CORE]
        m = dict(consts)
        m["xT8"] = np.ascontiguousarray(xc.T.astype(F16))  # [8, 65536]
        in_maps.append(m)
    return in_maps


def reconstruct(results):
    outs = []
    for r in results:
        od = np.asarray(r["out"])                       # [128, 32768] fp16
        a = od.astype(np.float32).reshape(2, 64, NT, 512)  # (h, f, t, i)
        a = a.transpose(0, 2, 3, 1)                        # (h, t, i, f)
        outs.append(a.reshape(ROWS_CORE, OUT))             # row = 32768h+512t+i
    return np.concatenate(outs, 0).reshape(B, S, OUT)


# ---------------------------------------------------------------------------
# bass module
# ---------------------------------------------------------------------------

def build_nc():
    import concourse.bass as bass
    import concourse.bacc as bacc
    import concourse.mybir as mybir
    import concourse.tile as tile

    f32 = mybir.dt.float32
    f16 = mybir.dt.float16
    Relu = mybir.ActivationFunctionType.Relu
    Ident = mybir.ActivationFunctionType.Identity
    Alu = mybir.AluOpType

    nc = bacc.Bacc(None, target_bir_lowering=False)

    xT8_d = nc.dram_tensor("xT8", [FEAT, ROWS_CORE], f16, kind="ExternalInput")
    a1t_d = nc.dram_tensor("a1t", [64, 96], f16, kind="ExternalInput")
    a2t_d = nc.dram_tensor("a2t", [96, 64], f16, kind="ExternalInput")
    wpt_d = nc.dram_tensor("wpt", [128, 64], f16, kind="ExternalInput")
    b1c_d = nc.dram_tensor("b1c", [96, 1], f32, kind="ExternalInput")
    b2c_d = nc.dram_tensor("b2c", [128, 1], f32, kind="ExternalInput")
    bpc_d = nc.dram_tensor("bpc", [128, 1], f32, kind="ExternalInput")
    out_d = nc.dram_tensor("out", [128, ROWS_CORE // 2], f16, kind="ExternalOutput")

    with tile.TileContext(nc) as tc:
        with (
            tc.tile_pool(name="consts", bufs=1) as cpool,
            tc.tile_pool(name="xin", bufs=2) as xpool,
            tc.tile_pool(name="h1s", bufs=3) as h1pool,
            tc.tile_pool(name="h2s", bufs=3) as h2pool,
            tc.tile_pool(name="osb", bufs=2) as opool,
            tc.tile_pool(name="ps_h1", bufs=2, space="PSUM") as ps_h1,
            tc.tile_pool(name="ps_h2", bufs=2, space="PSUM") as ps_h2,
            tc.tile_pool(name="ps_o", bufs=2, space="PSUM") as ps_o,
        ):
            a1t = cpool.tile([64, 96], f16)
            a2t = cpool.tile([96, 64], f16)
            wpt = cpool.tile([128, 64], f16)
            b1c = cpool.tile([96, 1], f32)
            b2c = cpool.tile([128, 1], f32)
            bpc = cpool.tile([128, 1], f32)
            nc.sync.dma_start(a1t[:], a1t_d[:])
            nc.sync.dma_start(a2t[:], a2t_d[:])
            nc.sync.dma_start(wpt[:], wpt_d[:])
            nc.sync.dma_start(b1c[:], b1c_d[:])
            nc.sync.dma_start(b2c[:], b2c_d[:])
            nc.sync.dma_start(bpc[:], bpc_d[:])

            def load_batch(T):
                xt = xpool.tile([64, TPB * 512], f16)
                for q in range(2):
                    nc.sync.dma_start(
                        xt[32 * q:32 * q + 8, :],
                        xT8_d[:, HALF * q + 4096 * T:HALF * q + 4096 * T + 4096],
                    )
                return xt

            xts = {0: load_batch(0)}
            h1ps_t = {}
            h1s_t = {}
            h2ps_t = {}
            h2s_t = {}
            outps_t = {}
            outsb_cur = {}

            # software-pipelined loop: PE does conv1(t), conv2(t-1), linear(t-2)
            for t in range(NT + 2):
                if t < NT:
                    T, s = t // TPB, t % TPB
                    if s == 0 and T + 1 < NB and (T + 1) not in xts:
                        xts[T + 1] = load_batch(T + 1)
                    xt = xts[T]
                    # ---- conv1(t): 2 strip-packed K=8 matmuls ----
                    h1ps = ps_h1.tile([96, 1024], f32)
                    for q in range(2):
                        nc.tensor.matmul(
                            h1ps[:, 512 * q:512 * q + 512],
                            a1t[32 * q:32 * q + 8, :],
                            xt[32 * q:32 * q + 8, 512 * s:512 * s + 512],
                            tile_position=(32 * q, 0),
                        )
                    h1ps_t[t] = h1ps
                    # ---- relu1 + b1 (DVE) ----
                    h1s = h1pool.tile([96, 1024], f16)
                    nc.vector.tensor_scalar(
                        h1s[:], h1ps[:], b1c[:], 0.0, Alu.add, Alu.max
                    )
                    h1s_t[t] = h1s
                if 1 <= t <= NT:
                    u = t - 1
                    # ---- conv2(u): col-packed K=96 pair ----
                    h1s = h1s_t.pop(u)
                    h2ps = ps_h2.tile([128, 512], f32)
                    nc.tensor.matmul(
                        h2ps[0:64, :], a2t[:], h1s[:, 0:512],
                        tile_position=(0, 0),
                    )
                    nc.tensor.matmul(
                        h2ps[64:128, :], a2t[:], h1s[:, 512:1024],
                        tile_position=(0, 64),
                    )
                    h2ps_t[u] = h2ps
                    # ---- relu2 + b2 (ACT) ----
                    h2s = h2pool.tile([128, 512], f16)
                    nc.scalar.activation(h2s[:], h2ps[:], Relu, bias=b2c[:])
                    h2s_t[u] = h2s
                if 2 <= t:
                    u = t - 2
                    T, s = u // TPB, u % TPB
                    if s == 0:
                        outsb_cur[0] = opool.tile(
                            [128, TPB * 512], f16, name="outsb"
                        )
                    outsb = outsb_cur[0]
                    # ---- linear(u): (0,0)+(64,64) pair, K=64 ----
                    h2s = h2s_t.pop(u)
                    outps = ps_o.tile([128, 512], f32)
                    nc.tensor.matmul(
                        outps[0:64, :], wpt[0:64, :], h2s[0:64, :],
                        tile_position=(0, 0),
                    )
                    nc.tensor.matmul(
                        outps[64:128, :], wpt[64:128, :], h2s[64:128, :],
                        tile_position=(64, 64),
                    )
                    # ---- out + bp (ACT) ----
                    nc.scalar.activation(
                        outsb[:, 512 * s:512 * s + 512], outps[:], Ident,
                        bias=bpc[:],
                    )
                    if s == TPB - 1:
                        nc.sync.dma_start(
                            out_d[:, 4096 * T:4096 * T + 4096], outsb[:]
                        )

    nc.compile()
    return nc


# ---------------------------------------------------------------------------
# entry point
# ---------------------------------------------------------------------------

_CACHE = {}


def _get_nc():
    if "nc" not in _CACHE:
        _CACHE["nc"] = build_nc()
    return _CACHE["nc"]


def kernel(x, W1, b1, W2, b2, Wp, bp):
    from concourse.bass_utils import run_bass_kernel_spmd

    nc = _get_nc()
    in_maps = build_in_maps(x, W1, b1, W2, b2, Wp, bp)
    res = run_bass_kernel_spmd(nc, in_maps, core_ids=list(range(NCORES)))
    return reconstruct(res.results)
